# revision 51
# baseline (speedup 1.0000x reference)
"""Trainium2 Bass kernel for nn_Head (single-head causal self-attention).

Module:  q = x@Wq.T, k = x@Wk.T, v = x@Wv.T
         wei = softmax(causal_mask(q@k.T * E**-0.5))
         out = wei @ v
Shapes:  x [2048, 128, 192], Wq/Wk/Wv [192, 192] -> out [2048, 128, 192]

Strategy (pure data parallel over the batch dim, 8 cores x 256 batches):
  - Weight fold: wei = x @ A @ x.T with A = (Wq.T @ Wk) * SCALE, so only one
    projection ("gT = A.T @ xT") is needed for the attention logits.
  - weiT[k,q] = sum_e xT[e,k] gT[e,q] computed DIRECTLY in transposed
    orientation (lhsT = xT slice), eliminating the PE transposes the
    previous version needed.
  - PmT = exp(weiT) (ACT) * causal_maskT (DVE).  v = x @ Wv.T per batch.
  - o_ext = PmT.T @ [v | 1]: one matmul yields both the unnormalized output
    and the softmax denominator (ones column).
  - NO on-device normalization: o_ext is copied PSUM->SBUF as bf16 (plain
    copy, no reciprocal / per-batch scalar ops) and DMA'd to DRAM in
    [T, nb, H+1] bf16 layout; the host divides by the denominator and
    transposes to [nb, T, H].  Output DMA bytes are halved vs f32.
  - ALL contractions zero-padded to K=128: sub-128-K matmuls measured ~1.8x
    slower per matmul on TRN2 (they defeat the warm-clock/LDW pipelining);
    with the padding the PE sustains ~100ns/matmul (2.4GHz, LDW hidden).
  - Engine split: ACT: exp + both gsb copies (single-engine wait for the
    weiT matmuls); DVE: v copy + o copy; Pool: causal mask (affine_select).
"""

import os
import sys

sys.path.insert(0, "/opt/trn_rl_repo")

import numpy as np
import ml_dtypes
from contextlib import ExitStack

import json

import concourse.bass as bass
import concourse.bass2jax as bass2jax
import concourse.mybir as mybir
import concourse.tile as tile
from concourse.bass_utils import (
    compile_bir_kernel as _orig_compile_bir_kernel,
    run_bass_kernel_spmd,
)

BF16 = mybir.dt.bfloat16
F32 = mybir.dt.float32
NPBF16 = ml_dtypes.bfloat16

B, T, E, H = 2048, 128, 192, 192
NCORES = 8
NB = B // NCORES            # batches per core
SCALE = float(E) ** -0.5
G = 8                       # batches per DMA group
QUAD = 4                    # batches per pipeline iteration
NGROUPS = NB // G


def _patch_tile_tail_drain():
    """Walrus rejects the TileContext tail Drain when it carries more than a
    couple of sem waits ("Too many sync wait commands").  Redistribute the
    waits onto single-wait SP nops emitted between the drain and barrier."""
    if getattr(tile.TileContext, "_tail_drain_patched", False):
        return

    def _drain_and_barrier(self, tick_clock, wait_clock):
        from concourse.tile import ScopedClock

        drain_inst = self.nc.sync.drain()
        wait_clock.add_sem_waits(
            drain_inst.ins, ScopedClock({None: tick_clock.global_clock})
        )
        waits = list(drain_inst.ins.sync_info.on_wait or [])
        if len(waits) > 1:
            drain_inst.ins.sync_info = mybir.SyncInfo(
                on_wait=[waits[0]], on_update=[]
            )
            for w in waits[1:]:
                nop = self.nc.sync.nop()
                nop.ins.sync_info = mybir.SyncInfo(on_wait=[w], on_update=[])
        self.nc.all_engine_barrier()
        assert self.sems is not None
        popped = self.nc._tile_sem_poison_stack.pop()
        assert popped is self._sem_poison
        self.nc.clear_and_free_semaphores(list(self.sems.allocated().values()))
        self.nc.all_engine_barrier()

    tile.TileContext._drain_and_barrier = _drain_and_barrier
    tile.TileContext._tail_drain_patched = True


def _split_multi_waits(bir_json: bytes) -> bytes:
    """This container's walrus supports only ONE sync-wait slot per
    instruction ("Too many sync wait commands").  Hoist extra waits onto
    single-wait NoOps inserted just before the instruction (same engine, so
    per-engine program order and blocking semantics are preserved)."""
    d = json.loads(bir_json)
    n = 0
    for f in d.get("functions", []):
        for bb in f.get("blocks", []):
            insts = bb.get("instructions", [])
            out = []
            changed = False
            for inst in insts:
                si = inst.get("sync_info")
                waits = (si.get("on_wait") or []) if si else []
                if len(waits) > 1:
                    changed = True
                    for w in waits[:-1]:
                        n += 1
                        out.append({
                            "debug": inst.get("debug"),
                            "engine": inst["engine"],
                            "ins": [],
                            "name": f"WSPLIT-{n}",
                            "opcode": "NoOp",
                            "outs": [],
                            "sync_info": {"on_update": [], "on_wait": [w]},
                        })
                    si["on_wait"] = [waits[-1]]
                out.append(inst)
            if changed:
                bb["instructions"] = out
    if n == 0:
        return bir_json
    return json.dumps(d).encode()


def _patched_compile_bir_kernel(bir_json, tmpdir, neff_name="file.neff"):
    if isinstance(bir_json, str):
        bir_json = bir_json.encode()
    return _orig_compile_bir_kernel(_split_multi_waits(bir_json), tmpdir, neff_name)


bass2jax.compile_bir_kernel = _patched_compile_bir_kernel

# Enable walrus' redundant-LDWEIGHTS elision: consecutive matmuls that share
# the same stationary operand (our weiT/v pairs) skip the reload.
import concourse.bass_utils as _bu_mod

_orig_run_command = _bu_mod.run_command


def _run_command_ldwopt(cmd, **kw):
    if isinstance(cmd, list) and os.environ.get("BASS_LDW_OPT", "0") == "1":
        cmd = [
            c.replace("--enable-ldw-opt=false", "--enable-ldw-opt=true")
            if isinstance(c, str) else c
            for c in cmd
        ]
    return _orig_run_command(cmd, **kw)


_bu_mod.run_command = _run_command_ldwopt


def build_nc(nb=NB):
    _patch_tile_tail_drain()
    nc = bass.Bass(trn_type="TRN2")

    # All contractions are zero-padded to K=128 (sub-128-K matmuls measure
    # ~1.8x slower per MM on this hardware).  xt rows: [x.T (192) | ones row
    # (for the softmax denominator) | zeros to 256].  A is padded to
    # [256, 256] so even the ghi output rows 65..127 are computed zeros.
    xt = nc.dram_tensor("xt", [2 * 128, nb * T], BF16, kind="ExternalInput")
    a = nc.dram_tensor("a", [2 * 128, 2 * 128], BF16, kind="ExternalInput")
    wvt = nc.dram_tensor("wvt", [2 * 128, H + 1], BF16, kind="ExternalInput")
    # Output: [T, nb, H+1] bf16 — unnormalized attention output plus the
    # softmax denominator in the last column; host normalizes + transposes.
    o = nc.dram_tensor("o", [T, nb, H + 1], BF16, kind="ExternalOutput")

    nq = nb // QUAD

    with tile.TileContext(nc) as tc, ExitStack() as ctx:
        singles = ctx.enter_context(tc.tile_pool(name="singles", bufs=1))
        px = ctx.enter_context(tc.tile_pool(name="px", bufs=4))
        pgsb = ctx.enter_context(tc.tile_pool(name="pgsb", bufs=2))
        pp = ctx.enter_context(tc.tile_pool(name="pp", bufs=3))
        ppm = ctx.enter_context(tc.tile_pool(name="ppm", bufs=3))
        pvsb = ctx.enter_context(tc.tile_pool(name="pvsb", bufs=6))
        posb = ctx.enter_context(tc.tile_pool(name="posb", bufs=3))

        pg = ctx.enter_context(tc.tile_pool(name="pg", bufs=1, space="PSUM"))
        pw = ctx.enter_context(tc.tile_pool(name="pw", bufs=2, space="PSUM"))
        pv = ctx.enter_context(tc.tile_pool(name="pv", bufs=1, space="PSUM"))
        po = ctx.enter_context(tc.tile_pool(name="po", bufs=1, space="PSUM"))

        # Constants: A (lhsT for gT), WvT_ext (rhs for v).  Declared before
        # the loop but DMA'd after the first x-group so the critical-path
        # group-0 transfer hits the queues first.
        a_lo = singles.tile([128, 256], BF16, tag="a_lo")
        a_hi = singles.tile([128, 256], BF16, tag="a_hi")
        wvt_lo = singles.tile([128, H + 1], BF16, tag="wvt_lo")
        wvt_hi = singles.tile([128, H + 1], BF16, tag="wvt_hi")

        # Software pipeline over quads Q:
        #   iter Q emits: x-DMA prefetch, gT(Q), gsb copies(Q) [ACT],
        #                 weiT/v(Q-1), exp(Q-1) [ACT], mask(Q-1) [Pool],
        #                 v copies(Q-1) [DVE], o(Q-2) + copy [DVE] + DMA out.
        x_tiles = {}     # group -> (xlo, xhi)
        gsb_t = {}       # Q -> (gsb_lo, gsb_hi)
        pm_t = {}        # Q -> pm (masked exp(weiT), bf16 SBUF)
        vsb_t = {}       # Q -> [v_sb pair0, v_sb pair1]

        def dma_in_group(g, halves=1):
            gcol = g * G * T
            xlo = px.tile([128, G * T], BF16, tag="xlo", name="xlo")
            xhi = px.tile([128, G * T], BF16, tag="xhi", name="xhi")
            hw_ = G * T // halves
            for hh in range(halves):
                hs, he = hh * hw_, (hh + 1) * hw_
                nc.sync.dma_start(out=xlo[:, hs:he],
                                  in_=xt[0:128, gcol + hs : gcol + he])
                nc.sync.dma_start(out=xhi[:, hs:he],
                                  in_=xt[128:256, gcol + hs : gcol + he])
            x_tiles[g] = (xlo, xhi)

        # Group 0 split so the first gT matmul only waits on the first
        # slice (subtile deps); DMAs ordered by first consumer: the very
        # first matmul needs only xlo[:, 0:Q1] and a_lo.
        xlo0_t = px.tile([128, G * T], BF16, tag="xlo", name="xlo")
        xhi0_t = px.tile([128, G * T], BF16, tag="xhi", name="xhi")
        Q1 = QUAD * T
        # consts go out via the idle Pool engine's software DGE (cheap
        # dispatch) so SP's queue is free for the critical x transfers
        nc.gpsimd.dma_start(out=a_lo, in_=a[0:128, :])
        nc.gpsimd.dma_start(out=a_hi, in_=a[128:256, :])
        nc.gpsimd.dma_start(out=wvt_lo, in_=wvt[0:128, :])
        nc.gpsimd.dma_start(out=wvt_hi, in_=wvt[128:256, :])
        nc.sync.dma_start(out=xlo0_t[:, 0:Q1], in_=xt[0:128, 0:Q1])
        nc.sync.dma_start(out=xhi0_t[:, 0:Q1], in_=xt[128:256, 0:Q1])
        nc.sync.dma_start(out=xlo0_t[:, Q1:], in_=xt[0:128, Q1 : G * T])
        nc.sync.dma_start(out=xhi0_t[:, Q1:], in_=xt[128:256, Q1 : G * T])
        x_tiles[0] = (xlo0_t, xhi0_t)
        dma_in_group(1)

        for Q in range(nq + 2):
            if Q < nq:
                g = Q * QUAD // G
                if (Q * QUAD) % G == 0 and g + 2 < NGROUPS:
                    dma_in_group(g + 2)  # prefetch two groups ahead
                xlo, xhi = x_tiles[g]
                qs = (Q * QUAD * T) % (G * T)
                qcols = slice(qs, qs + QUAD * T)

                # gT = A.T @ xT for 4 batches (N=512); all K=128
                glo = pg.tile([128, QUAD * T], F32, tag="glo")
                ghi = pg.tile([128, QUAD * T], F32, tag="ghi")
                nc.tensor.matmul(glo, a_lo[:, 0:128], xlo[:, qcols],
                                 start=True, stop=False)
                nc.tensor.matmul(glo, a_hi[:, 0:128], xhi[:, qcols],
                                 start=False, stop=True)
                nc.tensor.matmul(ghi, a_lo[:, 128:256], xlo[:, qcols],
                                 start=True, stop=False)
                nc.tensor.matmul(ghi, a_hi[:, 128:256], xhi[:, qcols],
                                 start=False, stop=True)
                # both gsb copies on ACT so the weiT matmuls wait on a
                # single engine clock
                gsb_lo = pgsb.tile([128, QUAD * T], BF16, tag="gsb_lo")
                gsb_hi = pgsb.tile([128, QUAD * T], BF16, tag="gsb_hi")
                nc.scalar.copy(out=gsb_lo, in_=glo)
                nc.scalar.copy(out=gsb_hi, in_=ghi)
                gsb_t[Q] = (gsb_lo, gsb_hi)

            if 1 <= Q <= nq:
                q0 = Q - 1
                g0 = q0 * QUAD // G
                xlo0, xhi0 = x_tiles[g0]
                qs0 = (q0 * QUAD * T) % (G * T)
                gsb_lo, gsb_hi = gsb_t.pop(q0)

                # weiT[k, j, q] and v_ext per batch; adjacent matmuls share
                # the same stationary operand (xlo/xhi slice).  The v_hi
                # matmul includes the ones row -> v_ext[:, H] = 1.
                wei = pw.tile([128, QUAD, T], F32, tag="wei")
                # v for all 4 batches in one 2-bank tile (256-float stride
                # keeps each matmul output inside one PSUM bank)
                v_ps = pv.tile([128, QUAD, 256], F32, tag="v_ps")
                for j in range(QUAD):
                    bs = qs0 + j * T
                    jc = slice(j * T, (j + 1) * T)
                    nc.tensor.matmul(wei[:, j, :], xlo0[:, bs : bs + T],
                                     gsb_lo[:, jc], start=True, stop=False)
                    nc.tensor.matmul(v_ps[:, j, 0 : H + 1],
                                     xlo0[:, bs : bs + T],
                                     wvt_lo, start=True, stop=False)
                    nc.tensor.matmul(wei[:, j, :], xhi0[:, bs : bs + T],
                                     gsb_hi[:, jc], start=False, stop=True)
                    nc.tensor.matmul(v_ps[:, j, 0 : H + 1],
                                     xhi0[:, bs : bs + T],
                                     wvt_hi, start=False, stop=True)

                # PmT = causal_mask(exp(weiT)): exp on ACT, mask on Pool
                p_sb = pp.tile([128, QUAD, T], BF16, tag="p_sb")
                nc.scalar.activation(out=p_sb, in_=wei,
                                     func=mybir.ActivationFunctionType.Exp)
                pm = ppm.tile([128, QUAD, T], BF16, tag="pm")
                nc.gpsimd.affine_select(
                    out=pm, in_=p_sb,
                    compare_op=mybir.AluOpType.is_ge,
                    fill=0.0, base=0, pattern=[[0, QUAD], [1, 128]],
                    channel_multiplier=-1,
                )
                pm_t[q0] = pm

                # v_ext -> SBUF (bf16); one strided DVE copy for the quad
                v_sb = pvsb.tile([128, QUAD, H + 1], BF16, tag="v_sb")
                nc.vector.tensor_copy(out=v_sb, in_=v_ps[:, :, 0 : H + 1])
                vsb_t[q0] = v_sb

            if Q >= 2:
                q2 = Q - 2
                pm = pm_t.pop(q2)
                v_sb2 = vsb_t.pop(q2)
                # padded to 256 floats per batch so each matmul output stays
                # inside one 2KB PSUM bank
                o_ps = po.tile([128, QUAD, 256], F32, tag="o_ps")
                for j in range(QUAD):
                    nc.tensor.matmul(o_ps[:, j, 0 : H + 1], pm[:, j, :],
                                     v_sb2[:, j, :],
                                     start=True, stop=True)
                o_sb = posb.tile([128, QUAD, H + 1], BF16, tag="o_sb")
                nc.vector.tensor_copy(out=o_sb, in_=o_ps[:, :, 0 : H + 1])
                b0 = q2 * QUAD
                nc.sync.dma_start(out=o[:, b0 : b0 + QUAD, :], in_=o_sb)
    return nc


_cached = {}


def _get_nc(nb):
    if nb not in _cached:
        _cached[nb] = build_nc(nb)
    return _cached[nb]


def prep_inputs(x, Wq, Wk, Wv, nb=NB, ncores=NCORES):
    """Host-side sharding + layout/dtype prep + weight folding."""
    x = np.asarray(x, dtype=np.float32)
    A = (np.asarray(Wq, np.float32).T @ np.asarray(Wk, np.float32)) * SCALE
    # A padded to [256, 256]: zero rows/cols make every contraction K=128
    # and make the ghi output rows beyond 64 computed zeros.
    a_ext = np.zeros((256, 256), np.float32)
    a_ext[0:E, 0:E] = A
    a_bf = a_ext.astype(NPBF16)
    # wvt_ext: [256, H+1] — Wv.T padded with a ones corner (row E, col H) so
    # the v matmul also produces the softmax denominator column; zeros below.
    wvt_ext = np.zeros((256, H + 1), np.float32)
    wvt_ext[0:E, 0:H] = np.asarray(Wv, np.float32).T
    wvt_ext[E, H] = 1.0
    wvt_bf = wvt_ext.astype(NPBF16)
    in_maps = []
    for c in range(ncores):
        shard = x[c * nb : (c + 1) * nb]                      # [nb, T, E]
        xt = np.zeros((256, nb * T), np.float32)
        xt[0:E] = shard.transpose(2, 0, 1).reshape(E, nb * T)
        xt[E] = 1.0
        in_maps.append({"xt": xt.astype(NPBF16), "a": a_bf, "wvt": wvt_bf})
    return in_maps


def kernel(x, Wq, Wk, Wv, _trace=False):
    nc = _get_nc(NB)
    in_maps = prep_inputs(x, Wq, Wk, Wv)
    res = run_bass_kernel_spmd(
        nc, in_maps, core_ids=list(range(NCORES)), trace=_trace
    )
    parts = []
    for c in range(NCORES):
        oc = np.asarray(res.results[c]["o"], dtype=np.float32)  # [T, nb, H+1]
        num = oc[:, :, 0:H]
        den = oc[:, :, H : H + 1]
        parts.append(np.transpose(num / den, (1, 0, 2)))      # [nb, T, H]
    out = np.ascontiguousarray(np.concatenate(parts, axis=0))
    if _trace:
        kernel.last_result = res
    return out
